# revision 23
# baseline (speedup 1.0000x reference)
"""Trainium2 Bass kernel for nn_Attention (B=2, N=2048, C=768, H=12, D=64).

Sharding: 8 cores = 2 batches x 4 head-groups (3 heads each).
Per core: full attention for its (batch, 3 heads) + row-sharded proj
partial output [2048, 768]; host sums the 4 partials per batch (+b_proj).

v4 design (vs v2 245us, v3 188us): ACT (exp) is the hard floor at
~101us (12.6M exp elems @ 1 elem/cyc/lane @ 1.2GHz + instr overhead).
Everything else is scheduled to hide under it:
  - All inputs arrive HOST-PACKED so every DMA is one contiguous
    per-partition line: x as [ch, p, ct, n] chunk-major bf16 (6KB
    lines), weights tile-packed bf16. No on-chip transposes or casts.
  - w_qk packed [k0,q0,k1,q1,k2,q2] -> the first score pair only needs
    m-tile t0; scores for head h emitted right after t_h's drains.
  - v computed in flipped orientation (out[tok, d] = xT_tile.T @ w_v)
    directly into the PV lhsT layout; bias via the PSUM->SBUF drain
    (tensor_tensor add with a host-replicated bias tile).
  - Scores: bf16, same-head k-tile (even,odd) pairs at PE row groups
    (0,0)/(64,0) run CONCURRENTLY on the row-tiled PE; q/k stored
    duplicated [128, N] (rows 0:64 == 64:128) via half-drains +
    SBUF->SBUF dup DMAs.
  - exp on ACT (scale=0.125 folded), [128,2,512] PSUM tiles -> bf16 P.
  - PV: lhsT = v planes [128, 65] (col 64 = ones -> softmax denominator
    for free), rhs = P planes; s_acc [65, 512] row 64 = denom.
  - Softmax denom: 3 heads' denom rows gathered (tiny SBUF DMAs) to
    partitions 0/32/64 of one tile -> ONE batched DVE reciprocal per
    q-chunk (reciprocal cost scales with free dim only, so batching by
    partition is 3x cheaper; reciprocal_approx_fast is broken on HW).
  - FINE-GRAINED INTERLEAVE: PE engine queues are in-order, so PV/proj
    matmuls are emitted BETWEEN score pairs.  While the 3-deep sc PSUM
    ring gates the next pair on ACT, the PE chews the interleaved
    filler instead of idling, and no backlog accumulates into a tail.
"""

import ml_dtypes
import numpy as np

import concourse.bass as bass
import concourse.mybir as mybir
from concourse import bacc, tile
from concourse.bass_utils import run_bass_kernel_spmd

F32 = mybir.dt.float32
F32R = mybir.dt.float32r
BF16 = mybir.dt.bfloat16
AF = mybir.ActivationFunctionType

B, N, C = 2, 2048, 768
H, D = 12, 64
SCALE = D ** -0.5  # 0.125
NCORES = 8
HPC = 3            # heads per core
NK = N // 128      # 16 k-tiles
NKP = NK // 2      # 8 k-tile pairs
NQ4 = N // 512     # 4 q-chunks of 512
CT = C // 128      # 6 c-tiles


def build_program():
    nc = bacc.Bacc("TRN2", target_bir_lowering=False, debug=False,
                   num_devices=NCORES)
    # x: [ch, p, ct, n] chunk-major so each chunk DMA is one 6KB line
    # per partition.  w_qk/wv tile-packed [p, ct, m] likewise.
    x_d = nc.dram_tensor("x", [NQ4 * 128, CT * 512], BF16,
                         kind="ExternalInput")
    wqk_d = nc.dram_tensor("wqk", [128, CT * 384], BF16,
                           kind="ExternalInput")
    wv_d = nc.dram_tensor("wv", [128, CT * 192], BF16,
                          kind="ExternalInput")
    bq_d = nc.dram_tensor("bq", [128, 3], F32, kind="ExternalInput")
    vbb_d = nc.dram_tensor("vbb", [128, 192], F32, kind="ExternalInput")
    wp_d = nc.dram_tensor("wp", [HPC * 64, C], BF16, kind="ExternalInput")
    y_d = nc.dram_tensor("y", [N, C], BF16, kind="ExternalOutput")

    with tile.TileContext(nc) as tc:
        with (
            tc.tile_pool(name="const", bufs=1) as cpool,
            tc.tile_pool(name="wr", bufs=1) as wrpool,
            tc.tile_pool(name="qk", bufs=1) as qkpool,
            tc.tile_pool(name="vn", bufs=1) as vnpool,
            tc.tile_pool(name="outT", bufs=1) as opool,
            tc.tile_pool(name="pt", bufs=44) as ptpool,
            tc.tile_pool(name="scps", bufs=2, space="PSUM") as scpool,
            tc.tile_pool(name="rc", bufs=2) as rcpool,
            tc.tile_pool(name="y", bufs=2) as ypool,
        ):
            vcol_f = cpool.tile([128, NKP, 2, HPC, 1], F32)
            ones_b = cpool.tile([65, 64], BF16)
            bq_sb = cpool.tile([128, 3], F32)
            vbb_sb = cpool.tile([128, 192], F32)

            w_qk = wrpool.tile([128, CT, 384], BF16)
            wv_sb = wrpool.tile([128, CT, 192], BF16)
            wp01 = wrpool.tile([128, C], BF16)
            wp1lo = wrpool.tile([64, C], BF16)  # h1 proj rows at base 0
            wp2 = wrpool.tile([64, C], BF16)

            # cross-head packing: h0 lives at partitions 0:64, h1 at
            # 64:128 (zero duplication); h2 at 0:64 with one q dup/chunk
            kA = qkpool.tile([128, N], BF16, tag="kA", name="kA")
            qA = qkpool.tile([128, N], BF16, tag="qA", name="qA")
            kqB = qkpool.tile([128, N], BF16, tag="kqB", name="kqB")
            q2lo = qkpool.tile([64, N], BF16, tag="q2lo", name="q2lo")
            # v planes: [k-part, pair, plane, head, 65] col 64 = ones
            v_n = vnpool.tile([128, NKP, 2, HPC, 65], BF16)

            # proj lhsT: pack01 = [outT_h0; outT_h1], h2 separate
            pack01 = opool.tile([128, N], BF16, tag="pk", name="pack01")
            outT1 = opool.tile([64, N], BF16, tag="o1", name="outT1")
            outT2 = opool.tile([64, N], BF16, tag="o2", name="outT2")

            pts = {}    # (qc, h, ktile) -> (P tile, plane)
            tidx = {}   # (qc, h, ktile) -> sc tile counter at emission
            s_sbs = {}  # h -> live s_sb snapshot of the current norm qc
            scur = {"sc": None, "pt": None, "plane": 0, "n": 0}

            def emit_unit(h, qc, kt):
                """One score matmul for (head, q-chunk, k-tile) into the
                next free plane of the current 2-plane sc tile; fires the
                exp as soon as the tile fills.  h0 at rows 0:64, h1 at
                64:128 (concurrent when adjacent), h2 at rows 0:64."""
                qs = slice(qc * 512, (qc + 1) * 512)
                ks = slice(kt * 128, (kt + 1) * 128)
                if scur["sc"] is None:
                    scur["sc"] = scpool.tile([128, 2, 512], F32, tag="sc",
                                             name="sc")
                    scur["pt"] = ptpool.tile([128, 2, 512], BF16, tag="pt",
                                             name="pt")
                    scur["plane"] = 0
                pl = scur["plane"]
                if h == 0:
                    nc.tensor.matmul(scur["sc"][:, pl, :], kA[0:64, ks],
                                     qA[0:64, qs], start=True, stop=True)
                elif h == 1:
                    nc.tensor.matmul(scur["sc"][:, pl, :], kA[64:128, ks],
                                     qA[64:128, qs], start=True, stop=True,
                                     tile_position=(64, 0))
                else:
                    nc.tensor.matmul(scur["sc"][:, pl, :], kqB[0:64, ks],
                                     q2lo[0:64, qs], start=True, stop=True)
                pts[(qc, h, kt)] = (scur["pt"], pl)
                tidx[(qc, h, kt)] = scur["n"]
                if pl == 1:
                    nc.scalar.activation(scur["pt"][:], scur["sc"][:],
                                         AF.Exp, scale=SCALE)
                    scur["sc"] = None
                    scur["pt"] = None
                    scur["n"] += 1
                else:
                    scur["plane"] = 1

            def emit_pv_kp(h, qc, kp, s_acc):
                """Two PV accumulation matmuls for k-tile pair kp."""
                pt0, pl0 = pts.pop((qc, h, 2 * kp))
                pt1, pl1 = pts.pop((qc, h, 2 * kp + 1))
                nc.tensor.matmul(s_acc[:], v_n[:, kp, 0, h, 0:65],
                                 pt0[:, pl0, :], start=(kp == 0), stop=False)
                nc.tensor.matmul(s_acc[:], v_n[:, kp, 1, h, 0:65],
                                 pt1[:, pl1, :], start=False,
                                 stop=(kp == NKP - 1))

            def emit_snap(h, qc, s_acc, dg):
                """Snapshot s_acc PSUM->SBUF (frees the bank fast) and DMA
                the denom row to partition 32h of the qc's gather tile."""
                s_sb = rcpool.tile([65, 512], F32, tag="ssb", name="s_sb",
                                   bufs=3)
                with tc.high_priority(offset=400):
                    nc.vector.tensor_copy(s_sb[:], s_acc[:])
                nc.gpsimd.dma_start(out=dg[32 * h:32 * h + 1, :],
                                    in_=s_sb[64:65, :])
                s_sbs[h] = s_sb

            def emit_norm_apply(qc, r, h, pjpool):
                """Broadcast 1/denom for one head and scale its o-block
                into the bf16 proj lhsT."""
                qs = slice(qc * 512, (qc + 1) * 512)
                bcs = pjpool.tile([128, 512], F32, tag="pj", name="bcs")
                nc.tensor.matmul(bcs[0:64, :],
                                 ones_b[32 * h:32 * h + 1, 0:64],
                                 r[32 * h:32 * h + 1, :],
                                 start=True, stop=True)
                if h == 0:
                    dst = pack01[0:64, qs]
                elif h == 1:
                    dst = outT1[0:64, qs]
                else:
                    dst = outT2[0:64, qs]
                nc.vector.tensor_mul(dst, s_sbs[h][0:64, :], bcs[0:64, :])
                if h == 1:
                    nc.sync.dma_start(out=pack01[64:128, qs],
                                      in_=outT1[0:64, qs])

            def emit_norm_qc(qc, dg, pjpool, j0=0, j1=4, dup=True):
                """One batched reciprocal for all 3 heads' denominators
                (partitions 0/32/64 of dg) over q columns [j0*128, j1*128),
                then per-head PE ones-matmul partition broadcast + multiply
                into the bf16 proj lhsT."""
                cs = slice(j0 * 128, j1 * 128)
                qs = slice(qc * 512 + j0 * 128, qc * 512 + j1 * 128)
                r = rcpool.tile([65, 512], BF16, tag="r", name="r")
                with nc.allow_low_precision(reason="softmax denom recip"):
                    with tc.high_priority(offset=400):
                        nc.vector.reciprocal(r[:, cs], dg[:, cs])
                w = (j1 - j0) * 128
                for h in range(HPC):
                    bcs = pjpool.tile([128, 512], F32, tag="pj", name="bcs")
                    nc.tensor.matmul(bcs[0:64, 0:w],
                                     ones_b[32 * h:32 * h + 1, 0:64],
                                     r[32 * h:32 * h + 1, cs],
                                     start=True, stop=True)
                    if h == 0:
                        dst = pack01[0:64, qs]
                    elif h == 1:
                        dst = outT1[0:64, qs]
                    else:
                        dst = outT2[0:64, qs]
                    nc.vector.tensor_mul(dst, s_sbs[h][0:64, cs],
                                         bcs[0:64, 0:w])
                    if dup and h == 1:
                        nc.sync.dma_start(out=pack01[64:128, qs],
                                          in_=outT1[0:64, qs])

            def emit_proj_j(qc, j, pjpool, nodup=False):
                qj = slice(qc * 512 + j * 128, qc * 512 + (j + 1) * 128)
                y_sb = ypool.tile([128, C], BF16, tag="y", name="ysb")
                pj = pjpool.tile([128, 512], F32, tag="pj", name="pj")
                if nodup:
                    # last-qc path: avoid waiting on the pack01 dup DMA by
                    # contracting the three heads as 64-deep matmuls
                    nc.tensor.matmul(pj[:], pack01[0:64, qj],
                                     wp01[0:64, 0:512],
                                     start=True, stop=False)
                    nc.tensor.matmul(pj[:], outT1[0:64, qj],
                                     wp1lo[0:64, 0:512],
                                     start=False, stop=False)
                else:
                    nc.tensor.matmul(pj[:], pack01[:, qj], wp01[:, 0:512],
                                     start=True, stop=False)
                nc.tensor.matmul(pj[:], outT2[0:64, qj], wp2[0:64, 0:512],
                                 start=False, stop=True)
                with tc.high_priority(offset=200):
                    nc.vector.tensor_copy(y_sb[:, 0:512], pj[:])
                pj2 = pjpool.tile([128, 512], F32, tag="pj", name="pj2")
                if nodup:
                    nc.tensor.matmul(pj2[:, 0:256], pack01[0:64, qj],
                                     wp01[0:64, 512:768],
                                     start=True, stop=False)
                    nc.tensor.matmul(pj2[:, 0:256], outT1[0:64, qj],
                                     wp1lo[0:64, 512:768],
                                     start=False, stop=False)
                else:
                    nc.tensor.matmul(pj2[:, 0:256], pack01[:, qj],
                                     wp01[:, 512:768], start=True,
                                     stop=False)
                nc.tensor.matmul(pj2[:, 0:256], outT2[0:64, qj],
                                 wp2[0:64, 512:768], start=False,
                                 stop=True)
                with tc.high_priority(offset=200):
                    nc.vector.tensor_copy(y_sb[:, 512:768], pj2[:, 0:256])
                dmaq = nc.gpsimd if (qc * 4 + j) % 2 else nc.sync
                dmaq.dma_start(out=y_d[qj, :], in_=y_sb[:])

            # ---------------- Phase 1 + scores(qc0) ----------------
            # qkv m-tiles (w packed [k0;k1],[q0;q1],[k2;q2]): each drain
            # is ONE [128,512] tensor_scalar; only h2's q needs a dup
            drain_plan = [kA, qA, kqB]
            with (
                tc.tile_pool(name="xT", bufs=2) as xtpool,
                tc.tile_pool(name="qps", bufs=2, space="PSUM") as qpspool,
            ):
                # weight loads first on the gpsimd DMA queue, in order of
                # first use; bq before wv (needed at the first drain)
                nc.gpsimd.dma_start(
                    out=w_qk[:],
                    in_=wqk_d.ap().rearrange("p (t m) -> p t m", t=CT))
                nc.gpsimd.dma_start(out=bq_sb[:], in_=bq_d[:])
                nc.gpsimd.dma_start(
                    out=wv_sb[:],
                    in_=wv_d.ap().rearrange("p (t m) -> p t m", t=CT))
                nc.gpsimd.dma_start(out=vbb_sb[:], in_=vbb_d[:])
                nc.gpsimd.dma_start(out=wp01[:], in_=wp_d[0:128, :])
                nc.gpsimd.dma_start(out=wp1lo[:], in_=wp_d[64:128, :])
                nc.gpsimd.dma_start(out=wp2[:], in_=wp_d[128:192, :])
                # HAM warmup: ~10 junk matmuls while the input DMAs land so
                # the PE clock is at 2.4GHz when real work arrives
                wrm = cpool.tile([64, 576], BF16, tag="wrm", name="wrm")
                nc.gpsimd.memset(wrm[:], 0.0)
                wps = qpspool.tile([128, 512], F32, tag="qkv", name="warm")
                for _ in range(10):
                    nc.tensor.matmul(wps[0:64, :], wrm[0:64, 0:64],
                                     wrm[0:64, 64:576], start=True,
                                     stop=True)

                for ch in range(NQ4):
                    ns = slice(ch * 512, (ch + 1) * 512)
                    xT = xtpool.tile([128, CT, 512], BF16, tag="xT",
                                     name=f"xT{ch}", bufs=2)
                    nc.sync.dma_start(
                        out=xT[:],
                        in_=x_d[ch * 128:(ch + 1) * 128, :]
                        .rearrange("p (t n) -> p t n", t=CT))
                    for t in range(3):
                        qps = qpspool.tile([128, 512], F32, tag="qkv",
                                           name=f"qps{t}_{ch}")
                        for ct in range(CT):
                            nc.tensor.matmul(qps[:],
                                             w_qk[:, ct,
                                                  t * 128:(t + 1) * 128],
                                             xT[:, ct, :], start=(ct == 0),
                                             stop=(ct == CT - 1))
                        with tc.high_priority(offset=400):
                            nc.vector.tensor_scalar(
                                drain_plan[t][:, ns], qps[:],
                                bq_sb[:, t:t + 1], None,
                                mybir.AluOpType.add)
                        if t == 1:
                            # h0/h1 pairs: feed ACT as soon as qA lands
                            for kt in range(4 * ch, 4 * ch + 4):
                                emit_unit(0, 0, kt)
                                emit_unit(1, 0, kt)
                        elif t == 2:
                            nc.gpsimd.dma_start(out=q2lo[0:64, ns],
                                                in_=kqB[64:128, ns])
                            for kt in range(4 * ch, 4 * ch + 4):
                                emit_unit(2, 0, kt)
                    if ch == 0:
                        # big inits kept off the gpsimd queue head so the
                        # first dup DMAs are not delayed behind them
                        nc.gpsimd.memset(v_n[:], 0.0)
                        nc.gpsimd.memset(vcol_f[:], 1.0)
                        nc.gpsimd.memset(ones_b[:], 1.0)
                        nc.vector.tensor_copy(v_n[:, :, :, :, 64:65],
                                              vcol_f[:])
                    # v in flipped orientation -> PV lhsT layout directly
                    for j in range(4):
                        ktile = ch * 4 + j
                        kp, pl = ktile // 2, ktile % 2
                        vps = qpspool.tile([128, 512], F32, tag="qkv",
                                           name=f"vps{ktile}")
                        for ct in range(CT):
                            nc.tensor.matmul(vps[:, 0:192],
                                             xT[:, ct, j * 128:(j + 1) * 128],
                                             wv_sb[:, ct, :], start=(ct == 0),
                                             stop=(ct == CT - 1))
                        nc.vector.tensor_add(
                            v_n[:, kp, pl, 0:HPC, 0:64],
                            vps[:, 0:192].rearrange("p (h d) -> p h d", h=3),
                            vbb_sb[:].rearrange("p (h d) -> p h d", h=3))
                    # qc1 lookahead so ACT stays fed into the steady state
                    if ch == 1:
                        for kt in range(4):
                            emit_unit(0, 1, kt)
                            emit_unit(1, 1, kt)
                    elif ch == 2:
                        for kt in range(4, 8):
                            emit_unit(0, 1, kt)
                            emit_unit(1, 1, kt)
                        for kt in range(8):
                            emit_unit(2, 1, kt)

            # ---------------- Steady state: qc 1..3 ----------------
            # PE queues are in-order: interleave PV/snap/norm/proj filler
            # between score-tile emissions so the sc-ring gate on ACT
            # never leaves the PE idle or defers work into a tail.
            # Fillers live in a FIFO; each carries an estimated PE cost
            # and a safe-after sc-tile count (ACT trails the PE score
            # stream by <= 3 ring tiles).  Each score tile pops ~0.84us
            # of filler so ACT is never starved by a filler-heavy block;
            # surplus flows to later blocks with slack.
            with (
                tc.tile_pool(name="accps", bufs=2, space="PSUM") as acpool,
                tc.tile_pool(name="pjps", bufs=2, space="PSUM") as pjpool,
            ):
                dgs = {}
                fifo = []  # (safe_after_tile_count, pe_cost_ns, closure)

                def push_pv_block(qc, h):
                    safe = tidx[(qc, h, NK - 1)] + 3
                    box = {}

                    def mk(kp):
                        def f():
                            if kp == 0:
                                box["acc"] = acpool.tile(
                                    [65, 512], F32, tag="acc",
                                    name=f"acc{qc}_{h}")
                            emit_pv_kp(h, qc, kp, box["acc"])
                        return f
                    for kp in range(NKP):
                        fifo.append((safe, 450, mk(kp)))

                    def snap():
                        if qc not in dgs:
                            dg = rcpool.tile([65, 512], F32, tag="dg",
                                             name="dg", bufs=2)
                            nc.gpsimd.memset(dg[:], 1.0)
                            dgs[qc] = dg
                        emit_snap(h, qc, box["acc"], dgs[qc])
                    fifo.append((safe, 100, snap))
                    if h == HPC - 1 and qc < NQ4 - 1:
                        box2 = {}

                        def recip_j(j):
                            def f():
                                if j == 0:
                                    box2["r"] = rcpool.tile(
                                        [65, 512], BF16, tag="r", name="r")
                                cs = slice(j * 128, (j + 1) * 128)
                                with nc.allow_low_precision(
                                        reason="softmax denom recip"):
                                    with tc.high_priority(offset=400):
                                        nc.vector.reciprocal(
                                            box2["r"][:, cs],
                                            dgs[qc][:, cs])
                            return f
                        for j in range(4):
                            fifo.append((safe, 250, recip_j(j)))

                        def apply_h(h2):
                            def f():
                                emit_norm_apply(qc, box2["r"], h2, pjpool)
                            return f
                        for h2 in range(HPC):
                            fifo.append((safe, 350, apply_h(h2)))
                        for j in range(4):
                            fifo.append((safe, 800,
                                         lambda j=j: emit_proj_j(qc, j,
                                                                 pjpool)))

                BUDGET = 840  # ns of filler PE time per score tile

                def pop_fillers(budget):
                    spent = 0
                    while fifo and fifo[0][0] <= scur["n"] and spent < budget:
                        _, cost, f = fifo.pop(0)
                        f()
                        spent += cost

                for h in range(HPC):
                    push_pv_block(0, h)
                for qc in range(1, NQ4):
                    last = qc == NQ4 - 1
                    kt0 = 8 if qc == 1 else 0  # qc1 kt0-7 pre-emitted
                    for kt in range(kt0, NK):
                        emit_unit(0, qc, kt)
                        emit_unit(1, qc, kt)
                        pop_fillers(1400 if last else BUDGET)
                    push_pv_block(qc, 0)
                    push_pv_block(qc, 1)
                    for kt in range(kt0, NK):
                        emit_unit(2, qc, kt)
                        pop_fillers(1400 if last else BUDGET)
                    push_pv_block(qc, 2)
                # tail: flush remaining fillers (PV(3,*) among them), then
                # the j-split pipelined endgame for qc3
                while fifo:
                    _, _, f = fifo.pop(0)
                    f()
                for j in range(4):
                    emit_norm_qc(NQ4 - 1, dgs[NQ4 - 1], pjpool, j0=j,
                                 j1=j + 1, dup=False)
                    emit_proj_j(NQ4 - 1, j, pjpool, nodup=True)
    nc.compile()
    return nc


def make_in_maps(x, w_qkv, b_qkv, w_proj):
    """Per-core input dicts. Core c: batch c//4, heads 3*(c%4)+[0..2]."""
    x = np.asarray(x, np.float32)
    w_qkv = np.asarray(w_qkv, np.float32)
    b_qkv = np.asarray(b_qkv, np.float32)
    w_proj = np.asarray(w_proj, np.float32)
    q = lambda h: w_qkv[:, h * 64:(h + 1) * 64]
    k = lambda h: w_qkv[:, C + h * 64: C + (h + 1) * 64]
    v = lambda h: w_qkv[:, 2 * C + h * 64: 2 * C + (h + 1) * 64]
    qb = lambda h: b_qkv[h * 64:(h + 1) * 64]
    kb = lambda h: b_qkv[C + h * 64: C + (h + 1) * 64]
    vb = lambda h: b_qkv[2 * C + h * 64: 2 * C + (h + 1) * 64]

    def tile_pack(w):  # [C, M] -> [128, CT*M] with [p, ct, m] layout
        M = w.shape[1]
        return np.ascontiguousarray(
            w.reshape(CT, 128, M).transpose(1, 0, 2).reshape(128, CT * M))

    in_maps = []
    for c in range(NCORES):
        b = c // 4
        h0 = 3 * (c % 4)
        hs = [h0, h0 + 1, h0 + 2]
        wqk = np.concatenate(
            [k(hs[0]), k(hs[1]), q(hs[0]), q(hs[1]), k(hs[2]), q(hs[2])],
            axis=1)  # [C, 384]
        wv = np.concatenate([v(hs[0]), v(hs[1]), v(hs[2])], axis=1)
        bqk = np.concatenate(
            [kb(hs[0]), kb(hs[1]), qb(hs[0]), qb(hs[1]), kb(hs[2]),
             qb(hs[2])])
        bq_pack = bqk.reshape(3, 128).T.copy()  # [128, 3]
        vbias = np.concatenate([vb(hs[0]), vb(hs[1]), vb(hs[2])])
        vbb = np.tile(vbias[None, :], (128, 1))  # [128, 192]
        wp_pack = np.concatenate(
            [w_proj[h * 64:(h + 1) * 64, :] for h in hs], axis=0)  # [192, C]
        # x chunk-major: [ch, p, ct, n] -> [NQ4*128, CT*512]
        xT = np.ascontiguousarray(x[b].T)  # [C, N]
        xp = xT.reshape(CT, 128, NQ4, 512).transpose(2, 1, 0, 3) \
            .reshape(NQ4 * 128, CT * 512)
        in_maps.append({
            "x": np.ascontiguousarray(xp).astype(ml_dtypes.bfloat16),
            "wqk": tile_pack(wqk).astype(ml_dtypes.bfloat16),
            "wv": tile_pack(wv).astype(ml_dtypes.bfloat16),
            "bq": np.ascontiguousarray(bq_pack.astype(np.float32)),
            "vbb": np.ascontiguousarray(vbb.astype(np.float32)),
            "wp": np.ascontiguousarray(wp_pack).astype(ml_dtypes.bfloat16),
        })
    return in_maps


_NC_CACHE = []


def _get_program():
    if not _NC_CACHE:
        _NC_CACHE.append(build_program())
    return _NC_CACHE[0]


def run(inputs, trace=False, **kw):
    nc = _get_program()
    in_maps = make_in_maps(inputs["x"], inputs["w_qkv"], inputs["b_qkv"],
                           inputs["w_proj"])
    res = run_bass_kernel_spmd(nc, in_maps, list(range(NCORES)), trace=trace,
                               **kw)
    b_proj = np.asarray(inputs["b_proj"], np.float32)
    out = np.zeros((B, N, C), np.float32)
    for c in range(NCORES):
        out[c // 4] += np.asarray(res.results[c]["y"]).astype(np.float32)
    out += b_proj[None, None, :]
    return out.astype(np.float32), res


def kernel(**inputs):
    out, _ = run(inputs)
    return out
